# revision 8
# baseline (speedup 1.0000x reference)
"""Trainium2 Bass kernel for the (non-standard) MultiHeadAttention module.

Reference math (B=4, N=2048, E=512, H=8):
    q/k/v  = x @ W{q,k,v} + b          # (B, N, E*H)
    split:   head h takes columns h::H  -> per-head (N, E) matrices
    attT_h = (k_h^T @ q_h) * 1/sqrt(N) # (f, e) -- attention over the E axis
    A_h    = exp(attT_h)               # softmax numerator (no max-sub needed,
                                       #  logits are O(±5))
    s_h[e] = sum_f A_h[f, e]
    y_h    = (A_h^T / s) @ v_h^T       # (e, n)
    final out row n' = 4e + (h//2) gets   y_{2r}[e] @ Wp[:2048] + y_{2r+1}[e] @ Wp[2048:] + bp
      (consequence of the reference's raw (B,E,H,N)->(B,N,E*H) reshape)

Key refactor: (A @ v^T) @ Wp_h == A @ (v^T @ Wp_h) = A @ P_h, which cuts FLOPs
and avoids transposing v.  Per-head bias bp/2 is folded into P_h, softmax
normalization applied at the very end:  out = U0*r0 + U1*r1 with
U_h = A_h^T @ (P_h + bp/2), r_h = 1/s_h.

Sharding: 16 independent units (b, r) with b in 0..3, r in 0..3; unit (b, r)
owns heads {2r, 2r+1} and produces output rows out[b, r::4, :].  Two units
per core, batch-major:  core c -> b = c//2, r in {2*(c%2), 2*(c%2)+1}.
No inter-core communication.

All matmuls run as float32r (fp32 storage, reduced-precision single-pass PE
mode: full speed for moving-free-dim >= 256).
"""

import numpy as np
from contextlib import ExitStack

import concourse.bass as bass
import concourse.mybir as mybir
import concourse.tile as tile
from concourse import bacc
from concourse.bass_utils import run_bass_kernel_spmd

B, N, E, H = 4, 2048, 512, 8
NT = N // 128          # 16 contraction chunks of 128 over n
EB = E // 128          # 4 blocks of 128 over e/f
SCALE = float(1.0 / np.sqrt(np.float32(N)))
F32 = mybir.dt.float32
F32R = mybir.dt.float32r
PSUM = bass.MemorySpace.PSUM

_CACHED_NC = None


def _bcast128(ap1d):
    """DMA access pattern replicating a 1-D DRAM row across 128 partitions."""
    return bass.AP(
        tensor=ap1d.tensor, offset=ap1d.offset, ap=[[0, 128]] + list(ap1d.ap)
    )


def build_nc():
    nc = bacc.Bacc("TRN2", target_bir_lowering=False, debug=False)

    xT_d = nc.dram_tensor("xT", (E, N), F32R, kind="ExternalInput")
    wq_d = nc.dram_tensor("wq", (2, 2, E, E), F32R, kind="ExternalInput")
    wk_d = nc.dram_tensor("wk", (2, 2, E, E), F32R, kind="ExternalInput")
    wv_d = nc.dram_tensor("wv", (2, 2, E, E), F32R, kind="ExternalInput")
    wp_d = nc.dram_tensor("wp", (2, N, E), F32R, kind="ExternalInput")
    bqkv_d = nc.dram_tensor("bqkv", (2, 2, 3, E), F32, kind="ExternalInput")
    bph_d = nc.dram_tensor("bph", (E,), F32, kind="ExternalInput")
    ones_d = nc.dram_tensor("ones", (128, 2), F32R, kind="ExternalInput")
    out_d = nc.dram_tensor("out", (2, E, E), F32, kind="ExternalOutput")

    with tile.TileContext(nc) as tc, ExitStack() as ctx:
        consts = ctx.enter_context(tc.tile_pool(name="consts", bufs=1))
        wp_pool = ctx.enter_context(tc.tile_pool(name="wp", bufs=2))
        wqkv_pool = ctx.enter_context(tc.tile_pool(name="wqkv", bufs=2))
        bias_pool = ctx.enter_context(tc.tile_pool(name="bias", bufs=1))
        qk_pool = ctx.enter_context(tc.tile_pool(name="qk", bufs=2))
        a_pool = ctx.enter_context(tc.tile_pool(name="a", bufs=2))
        p_pool = ctx.enter_context(tc.tile_pool(name="p", bufs=2))
        o_pool = ctx.enter_context(tc.tile_pool(name="o", bufs=1))
        r_pool = ctx.enter_context(tc.tile_pool(name="r", bufs=2))
        mm_ps = ctx.enter_context(tc.tile_pool(name="mmps", bufs=2, space=PSUM))
        big_ps = ctx.enter_context(tc.tile_pool(name="bigps", bufs=1, space=PSUM))
        u_ps = ctx.enter_context(tc.tile_pool(name="ups", bufs=2, space=PSUM))

        # Resident constants: x^T (E on partitions), ones column, bp/2 bcast.
        xt_sb = consts.tile([128, EB, N], F32R, tag="xt")
        nc.gpsimd.dma_start(
            out=xt_sb[:], in_=xT_d.ap().rearrange("(t p) n -> p t n", p=128)
        )
        ones_sb = consts.tile([128, 2], F32R, tag="ones")
        nc.gpsimd.dma_start(out=ones_sb[:], in_=ones_d.ap())
        bph_sb = consts.tile([128, E], F32, tag="bph")
        nc.gpsimd.dma_start(out=bph_sb[:], in_=_bcast128(bph_d.ap()))

        for u in range(2):
            A_tiles, P_tiles, R_tiles = [], [], []
            for hl in range(2):
                # --- weights + biases for head (u, hl) ---
                wq_sb = wqkv_pool.tile([128, EB, E], F32R, tag="wq")
                nc.gpsimd.dma_start(
                    out=wq_sb[:],
                    in_=wq_d.ap()[u, hl].rearrange("(t p) e -> p t e", p=128),
                )
                wk_sb = wqkv_pool.tile([128, EB, E], F32R, tag="wk")
                nc.gpsimd.dma_start(
                    out=wk_sb[:],
                    in_=wk_d.ap()[u, hl].rearrange("(t p) e -> p t e", p=128),
                )
                wv_sb = wqkv_pool.tile([128, EB, E], F32R, tag="wv")
                nc.gpsimd.dma_start(
                    out=wv_sb[:],
                    in_=wv_d.ap()[u, hl].rearrange("(t p) e -> p t e", p=128),
                )
                wp_sb = wp_pool.tile([128, NT, E], F32R, tag="wp")
                nc.gpsimd.dma_start(
                    out=wp_sb[:],
                    in_=wp_d.ap()[hl].rearrange("(t p) c -> p t c", p=128),
                )
                bq_sb = bias_pool.tile([128, E], F32, tag="bq")
                nc.gpsimd.dma_start(out=bq_sb[:], in_=_bcast128(bqkv_d.ap()[u, hl, 0]))
                bk_sb = bias_pool.tile([128, E], F32, tag="bk")
                nc.gpsimd.dma_start(out=bk_sb[:], in_=_bcast128(bqkv_d.ap()[u, hl, 1]))
                bv_sb = bias_pool.tile([128, E], F32, tag="bv")
                nc.gpsimd.dma_start(out=bv_sb[:], in_=_bcast128(bqkv_d.ap()[u, hl, 2]))

                # --- q/k projections fused with attT accumulation ---
                attT_ps = big_ps.tile([128, EB, E], F32, tag="big")
                for n in range(NT):
                    nsl = slice(n * 128, (n + 1) * 128)
                    q_ps = mm_ps.tile([128, E], F32, tag="mm")
                    for ec in range(EB):
                        nc.tensor.matmul(
                            q_ps[:],
                            xt_sb[:, ec, nsl],
                            wq_sb[:, ec, :],
                            start=ec == 0,
                            stop=ec == EB - 1,
                        )
                    q_sb = qk_pool.tile([128, E], F32R, tag="q")
                    nc.vector.tensor_add(q_sb[:], q_ps[:], bq_sb[:])
                    k_ps = mm_ps.tile([128, E], F32, tag="mm")
                    for ec in range(EB):
                        nc.tensor.matmul(
                            k_ps[:],
                            xt_sb[:, ec, nsl],
                            wk_sb[:, ec, :],
                            start=ec == 0,
                            stop=ec == EB - 1,
                        )
                    k_sb = qk_pool.tile([128, E], F32R, tag="k")
                    nc.vector.tensor_add(k_sb[:], k_ps[:], bk_sb[:])
                    for fb in range(EB):
                        nc.tensor.matmul(
                            attT_ps[:, fb, :],
                            k_sb[:, fb * 128 : (fb + 1) * 128],
                            q_sb[:],
                            start=n == 0,
                            stop=n == NT - 1,
                        )

                # --- exp (softmax numerator, transposed layout) ---
                A_sb = a_pool.tile([128, EB, E], F32R, tag="a")
                for fb in range(EB):
                    nc.scalar.activation(
                        out=A_sb[:, fb, :],
                        in_=attT_ps[:, fb, :],
                        func=mybir.ActivationFunctionType.Exp,
                        scale=SCALE,
                    )

                # --- s_e = sum_f A[f, e] via ones-matmuls; r = 1/s ---
                s_ps = mm_ps.tile([128, EB, 2], F32, tag="mm")
                for eb in range(EB):
                    esl = slice(eb * 128, (eb + 1) * 128)
                    for fc in range(EB):
                        nc.tensor.matmul(
                            s_ps[:, eb, :],
                            A_sb[:, fc, esl],
                            ones_sb[:],
                            start=fc == 0,
                            stop=fc == EB - 1,
                        )
                r_sb = r_pool.tile([128, EB, 2], F32, tag="r")
                nc.vector.reciprocal(out=r_sb[:], in_=s_ps[:])
                R_tiles.append(r_sb)

                # --- v projection fused with P = v^T @ Wp_h accumulation ---
                P_ps = big_ps.tile([128, EB, E], F32, tag="big")
                for n in range(NT):
                    nsl = slice(n * 128, (n + 1) * 128)
                    v_ps = mm_ps.tile([128, E], F32, tag="mm")
                    for ec in range(EB):
                        nc.tensor.matmul(
                            v_ps[:],
                            xt_sb[:, ec, nsl],
                            wv_sb[:, ec, :],
                            start=ec == 0,
                            stop=ec == EB - 1,
                        )
                    v_sb = qk_pool.tile([128, E], F32R, tag="v")
                    nc.vector.tensor_add(v_sb[:], v_ps[:], bv_sb[:])
                    for fb in range(EB):
                        nc.tensor.matmul(
                            P_ps[:, fb, :],
                            v_sb[:, fb * 128 : (fb + 1) * 128],
                            wp_sb[:, n, :],
                            start=n == 0,
                            stop=n == NT - 1,
                        )
                P_sb = p_pool.tile([128, EB, E], F32R, tag="p")
                for fb in range(EB):
                    nc.vector.tensor_add(P_sb[:, fb, :], P_ps[:, fb, :], bph_sb[:])
                A_tiles.append(A_sb)
                P_tiles.append(P_sb)

            # --- U_h = A_h^T @ P_h ; out = U0*r0 + U1*r1 ---
            out_sb = o_pool.tile([128, EB, E], F32, tag="o")
            for hl in range(2):
                for eb in range(EB):
                    esl = slice(eb * 128, (eb + 1) * 128)
                    u_tile = u_ps.tile([128, E], F32, tag="u")
                    for fc in range(EB):
                        nc.tensor.matmul(
                            u_tile[:],
                            A_tiles[hl][:, fc, esl],
                            P_tiles[hl][:, fc, :],
                            start=fc == 0,
                            stop=fc == EB - 1,
                        )
                    if hl == 0:
                        nc.vector.tensor_scalar_mul(
                            out_sb[:, eb, :], u_tile[:], R_tiles[0][:, eb, 0:1]
                        )
                    else:
                        nc.vector.scalar_tensor_tensor(
                            out_sb[:, eb, :],
                            u_tile[:],
                            R_tiles[1][:, eb, 0:1],
                            out_sb[:, eb, :],
                            op0=mybir.AluOpType.mult,
                            op1=mybir.AluOpType.add,
                        )
            nc.gpsimd.dma_start(
                out=out_d.ap()[u].rearrange("(t p) c -> p t c", p=128),
                in_=out_sb[:],
            )

    nc.compile()
    return nc


def _get_nc():
    global _CACHED_NC
    if _CACHED_NC is None:
        _CACHED_NC = build_nc()
    return _CACHED_NC


def make_in_maps(x, Wq, bq, Wk, bk, Wv, bv, Wp, bp):
    x = np.asarray(x, np.float32)
    Wq, Wk, Wv, Wp = (np.asarray(a, np.float32) for a in (Wq, Wk, Wv, Wp))
    bq, bk, bv, bp = (np.asarray(a, np.float32) for a in (bq, bk, bv, bp))
    wp_arr = np.ascontiguousarray(np.stack([Wp[:N], Wp[N:]]))
    bph = np.ascontiguousarray(0.5 * bp)
    in_maps = []
    for c in range(8):
        b = c // 2
        rs = [2 * (c % 2), 2 * (c % 2) + 1]
        heads = [[2 * r + hl for hl in range(2)] for r in rs]
        wq_arr = np.ascontiguousarray(
            np.stack([[Wq[:, h::H] for h in hu] for hu in heads])
        )
        wk_arr = np.ascontiguousarray(
            np.stack([[Wk[:, h::H] for h in hu] for hu in heads])
        )
        wv_arr = np.ascontiguousarray(
            np.stack([[Wv[:, h::H] for h in hu] for hu in heads])
        )
        bqkv = np.ascontiguousarray(
            np.stack([[[bq[h::H], bk[h::H], bv[h::H]] for h in hu] for hu in heads])
        )
        in_maps.append(
            {
                "xT": np.ascontiguousarray(x[b].T),
                "wq": wq_arr,
                "wk": wk_arr,
                "wv": wv_arr,
                "wp": wp_arr,
                "bqkv": bqkv,
                "bph": bph,
                "ones": np.ones((128, 2), np.float32),
            }
        )
    return in_maps


def assemble_out(results):
    out = np.empty((B, N, E), np.float32)
    for c in range(8):
        b = c // 2
        for ui in range(2):
            r = 2 * (c % 2) + ui
            out[b, r::4, :] = results[c]["out"][ui]
    return out


def run(inputs, trace=False, **spmd_kwargs):
    """Full pipeline; returns (output, BassKernelResults)."""
    nc = _get_nc()
    in_maps = make_in_maps(**inputs)
    res = run_bass_kernel_spmd(
        nc, in_maps, core_ids=list(range(8)), trace=trace, **spmd_kwargs
    )
    return assemble_out(res.results), res


def kernel(**inputs):
    out, _ = run(inputs)
    return out


# revision 9
# speedup vs baseline: 1.0779x; 1.0779x over previous
"""Trainium2 Bass kernel for the (non-standard) MultiHeadAttention module.

Reference math (B=4, N=2048, E=512, H=8):
    q/k/v  = x @ W{q,k,v} + b          # (B, N, E*H)
    split:   head h takes columns h::H  -> per-head (N, E) matrices
    attT_h = (k_h^T @ q_h) * 1/sqrt(N) # (f, e) -- attention over the E axis
    A_h    = exp(attT_h)               # softmax numerator (no max-sub needed,
                                       #  logits are O(+-5))
    s_h[e] = sum_f A_h[f, e]
    out row n' = 4e + r gets  sum_hl (A_h^T/s_h) @ (v_h^T @ Wp_half + bp/2)
      for h = 2r + hl  (consequence of the reference's raw
      (B,E,H,N)->(B,N,E*H) reshape before the output projection)

Key refactor: (A @ v^T) @ Wp_h == A @ (v^T @ Wp_h) = A @ P_h, which cuts FLOPs
and avoids transposing v.  Per-head bias bp/2 is folded into P_h, softmax
normalization applied at the very end:  out = U0*r0 + U1*r1 with
U_h = A_h^T @ (P_h + bp/2), r_h = 1/s_h.

Sharding: 16 independent units (b, r) with b in 0..3, r in 0..3; unit (b, r)
owns heads {2r, 2r+1} and produces output rows out[b, r::4, :].  Two units
per core, batch-major:  core c -> b = c//2, r in {2*(c%2), 2*(c%2)+1}.
No inter-core communication.

All matmuls run as float32r (fp32 storage, reduced-precision single-pass PE
mode: full speed for moving-free-dim >= 256).
"""

import numpy as np
from contextlib import ExitStack

import concourse.bass as bass
import concourse.mybir as mybir
import concourse.tile as tile
from concourse import bacc
from concourse.bass_utils import run_bass_kernel_spmd

B, N, E, H = 4, 2048, 512, 8
NT = N // 128          # 16 contraction chunks of 128 over n
EB = E // 128          # 4 blocks of 128 over e/f
SCALE = float(1.0 / np.sqrt(np.float32(N)))
F32 = mybir.dt.float32
F32R = mybir.dt.float32r
PSUM = bass.MemorySpace.PSUM

_CACHED_NC = None


def _bcast128(ap_nd):
    """DMA access pattern replicating a DRAM region across 128 partitions."""
    return bass.AP(
        tensor=ap_nd.tensor, offset=ap_nd.offset, ap=[[0, 128]] + list(ap_nd.ap)
    )


def build_nc():
    nc = bacc.Bacc("TRN2", target_bir_lowering=False, debug=False)

    xT_d = nc.dram_tensor("xT", (E, N), F32R, kind="ExternalInput")
    wq_d = nc.dram_tensor("wq", (2, 2, E, E), F32R, kind="ExternalInput")
    wk_d = nc.dram_tensor("wk", (2, 2, E, E), F32R, kind="ExternalInput")
    wv_d = nc.dram_tensor("wv", (2, 2, E, E), F32R, kind="ExternalInput")
    wp_d = nc.dram_tensor("wp", (2, N, E), F32R, kind="ExternalInput")
    bqkv_d = nc.dram_tensor("bqkv", (2, 2, 3, E), F32, kind="ExternalInput")
    bph_d = nc.dram_tensor("bph", (E,), F32, kind="ExternalInput")
    ones_d = nc.dram_tensor("ones", (128, 2), F32R, kind="ExternalInput")
    out_d = nc.dram_tensor("out", (2, E, E), F32, kind="ExternalOutput")

    with tile.TileContext(nc) as tc, ExitStack() as ctx:
        consts = ctx.enter_context(tc.tile_pool(name="consts", bufs=1))
        wp_pool = ctx.enter_context(tc.tile_pool(name="wp", bufs=8))
        wqkv_pool = ctx.enter_context(tc.tile_pool(name="wqkv", bufs=2))
        bias_pool = ctx.enter_context(tc.tile_pool(name="bias", bufs=2))
        qk_pool = ctx.enter_context(tc.tile_pool(name="qk", bufs=3))
        a_pool = ctx.enter_context(tc.tile_pool(name="a", bufs=2))
        p_pool = ctx.enter_context(tc.tile_pool(name="p", bufs=2))
        o_pool = ctx.enter_context(tc.tile_pool(name="o", bufs=2))
        r_pool = ctx.enter_context(tc.tile_pool(name="r", bufs=2))
        mm_ps = ctx.enter_context(tc.tile_pool(name="mmps", bufs=2, space=PSUM))
        big_ps = ctx.enter_context(tc.tile_pool(name="bigps", bufs=1, space=PSUM))
        u_ps = ctx.enter_context(tc.tile_pool(name="ups", bufs=2, space=PSUM))

        # Resident constants.  x^T is split into 4 per-E-chunk tiles so the
        # first projection matmuls only gate on the first 1MB DMA.
        xt_sb = []
        for ec in range(EB):
            t = consts.tile([128, N], F32R, tag=f"xt{ec}")
            nc.gpsimd.dma_start(
                out=t[:], in_=xT_d.ap()[ec * 128 : (ec + 1) * 128, :]
            )
            xt_sb.append(t)
        ones_sb = consts.tile([128, 2], F32R, tag="ones")
        nc.gpsimd.dma_start(out=ones_sb[:], in_=ones_d.ap())
        bph_sb = consts.tile([128, E], F32, tag="bph")
        nc.gpsimd.dma_start(out=bph_sb[:], in_=_bcast128(bph_d.ap()))

        for u in range(2):
            A_tiles, P_tiles, R_tiles = [], [], []
            for hl in range(2):
                # --- biases first (tiny, on the q/k copy critical path) ---
                bias_sb = bias_pool.tile([128, 3, E], F32, tag="bias")
                nc.gpsimd.dma_start(
                    out=bias_sb[:], in_=_bcast128(bqkv_d.ap()[u, hl])
                )
                bq_sb = bias_sb[:, 0, :]
                bk_sb = bias_sb[:, 1, :]
                bv_sb = bias_sb[:, 2, :]
                # --- q/k weights for head (u, hl) ---
                wq_sb = wqkv_pool.tile([128, EB, E], F32R, tag="wq")
                nc.gpsimd.dma_start(
                    out=wq_sb[:],
                    in_=wq_d.ap()[u, hl].rearrange("(t p) e -> p t e", p=128),
                )
                wk_sb = wqkv_pool.tile([128, EB, E], F32R, tag="wk")
                nc.gpsimd.dma_start(
                    out=wk_sb[:],
                    in_=wk_d.ap()[u, hl].rearrange("(t p) e -> p t e", p=128),
                )

                # --- q/k projections fused with attT accumulation ---
                attT_ps = big_ps.tile([128, EB, E], F32, tag="big")
                for n in range(NT):
                    nsl = slice(n * 128, (n + 1) * 128)
                    q_ps = mm_ps.tile([128, E], F32, tag="mm")
                    for ec in range(EB):
                        nc.tensor.matmul(
                            q_ps[:],
                            xt_sb[ec][:, nsl],
                            wq_sb[:, ec, :],
                            start=ec == 0,
                            stop=ec == EB - 1,
                        )
                    q_sb = qk_pool.tile([128, E], F32R, tag="q")
                    nc.vector.tensor_add(q_sb[:], q_ps[:], bq_sb)
                    k_ps = mm_ps.tile([128, E], F32, tag="mm")
                    for ec in range(EB):
                        nc.tensor.matmul(
                            k_ps[:],
                            xt_sb[ec][:, nsl],
                            wk_sb[:, ec, :],
                            start=ec == 0,
                            stop=ec == EB - 1,
                        )
                    k_sb = qk_pool.tile([128, E], F32R, tag="k")
                    nc.vector.tensor_add(k_sb[:], k_ps[:], bk_sb)
                    for fb in range(EB):
                        nc.tensor.matmul(
                            attT_ps[:, fb, :],
                            k_sb[:, fb * 128 : (fb + 1) * 128],
                            q_sb[:],
                            start=n == 0,
                            stop=n == NT - 1,
                        )

                # --- exp (softmax numerator, transposed layout) ---
                A_sb = a_pool.tile([128, EB, E], F32R, tag="a")
                for fb in range(EB):
                    nc.scalar.activation(
                        out=A_sb[:, fb, :],
                        in_=attT_ps[:, fb, :],
                        func=mybir.ActivationFunctionType.Exp,
                        scale=SCALE,
                    )

                # --- v projection fused with P = v^T @ Wp_h accumulation ---
                wv_sb = wqkv_pool.tile([128, EB, E], F32R, tag="wv")
                nc.gpsimd.dma_start(
                    out=wv_sb[:],
                    in_=wv_d.ap()[u, hl].rearrange("(t p) e -> p t e", p=128),
                )
                P_ps = big_ps.tile([128, EB, E], F32, tag="big")
                for n in range(NT):
                    nsl = slice(n * 128, (n + 1) * 128)
                    wp_sb = wp_pool.tile([128, E], F32R, tag="wp")
                    nc.gpsimd.dma_start(out=wp_sb[:], in_=wp_d.ap()[hl, nsl, :])
                    v_ps = mm_ps.tile([128, E], F32, tag="mm")
                    for ec in range(EB):
                        nc.tensor.matmul(
                            v_ps[:],
                            xt_sb[ec][:, nsl],
                            wv_sb[:, ec, :],
                            start=ec == 0,
                            stop=ec == EB - 1,
                        )
                    v_sb = qk_pool.tile([128, E], F32R, tag="v")
                    nc.vector.tensor_add(v_sb[:], v_ps[:], bv_sb)
                    for fb in range(EB):
                        nc.tensor.matmul(
                            P_ps[:, fb, :],
                            v_sb[:, fb * 128 : (fb + 1) * 128],
                            wp_sb[:],
                            start=n == 0,
                            stop=n == NT - 1,
                        )
                P_sb = p_pool.tile([128, EB, E], F32R, tag="p")
                for fb in range(EB):
                    nc.vector.tensor_add(P_sb[:, fb, :], P_ps[:, fb, :], bph_sb[:])

                # --- s_e = sum_f A[f, e] via ones-matmuls; r = 1/s ---
                # (emitted after the v/P loop so the PE doesn't stall on exp)
                s_ps = mm_ps.tile([128, EB, 2], F32, tag="mm")
                for eb in range(EB):
                    esl = slice(eb * 128, (eb + 1) * 128)
                    for fc in range(EB):
                        nc.tensor.matmul(
                            s_ps[:, eb, :],
                            A_sb[:, fc, esl],
                            ones_sb[:],
                            start=fc == 0,
                            stop=fc == EB - 1,
                        )
                r_sb = r_pool.tile([128, EB, 2], F32, tag="r")
                nc.vector.reciprocal(out=r_sb[:], in_=s_ps[:])
                R_tiles.append(r_sb)
                A_tiles.append(A_sb)
                P_tiles.append(P_sb)

            # --- U_h = A_h^T @ P_h ; out = U0*r0 + U1*r1, streamed per eb ---
            for eb in range(EB):
                esl = slice(eb * 128, (eb + 1) * 128)
                out_sb = o_pool.tile([128, E], F32, tag="o")
                for hl in range(2):
                    u_tile = u_ps.tile([128, E], F32, tag="u")
                    for fc in range(EB):
                        nc.tensor.matmul(
                            u_tile[:],
                            A_tiles[hl][:, fc, esl],
                            P_tiles[hl][:, fc, :],
                            start=fc == 0,
                            stop=fc == EB - 1,
                        )
                    if hl == 0:
                        nc.vector.tensor_scalar_mul(
                            out_sb[:], u_tile[:], R_tiles[0][:, eb, 0:1]
                        )
                    else:
                        nc.vector.scalar_tensor_tensor(
                            out_sb[:],
                            u_tile[:],
                            R_tiles[1][:, eb, 0:1],
                            out_sb[:],
                            op0=mybir.AluOpType.mult,
                            op1=mybir.AluOpType.add,
                        )
                nc.gpsimd.dma_start(
                    out=out_d.ap()[u, eb * 128 : (eb + 1) * 128, :],
                    in_=out_sb[:],
                )

    nc.compile()
    return nc


def _get_nc():
    global _CACHED_NC
    if _CACHED_NC is None:
        _CACHED_NC = build_nc()
    return _CACHED_NC


def make_in_maps(x, Wq, bq, Wk, bk, Wv, bv, Wp, bp):
    x = np.asarray(x, np.float32)
    Wq, Wk, Wv, Wp = (np.asarray(a, np.float32) for a in (Wq, Wk, Wv, Wp))
    bq, bk, bv, bp = (np.asarray(a, np.float32) for a in (bq, bk, bv, bp))
    wp_arr = np.ascontiguousarray(np.stack([Wp[:N], Wp[N:]]))
    bph = np.ascontiguousarray(0.5 * bp)
    in_maps = []
    for c in range(8):
        b = c // 2
        rs = [2 * (c % 2), 2 * (c % 2) + 1]
        heads = [[2 * r + hl for hl in range(2)] for r in rs]
        wq_arr = np.ascontiguousarray(
            np.stack([[Wq[:, h::H] for h in hu] for hu in heads])
        )
        wk_arr = np.ascontiguousarray(
            np.stack([[Wk[:, h::H] for h in hu] for hu in heads])
        )
        wv_arr = np.ascontiguousarray(
            np.stack([[Wv[:, h::H] for h in hu] for hu in heads])
        )
        bqkv = np.ascontiguousarray(
            np.stack([[[bq[h::H], bk[h::H], bv[h::H]] for h in hu] for hu in heads])
        )
        in_maps.append(
            {
                "xT": np.ascontiguousarray(x[b].T),
                "wq": wq_arr,
                "wk": wk_arr,
                "wv": wv_arr,
                "wp": wp_arr,
                "bqkv": bqkv,
                "bph": bph,
                "ones": np.ones((128, 2), np.float32),
            }
        )
    return in_maps


def assemble_out(results):
    out = np.empty((B, N, E), np.float32)
    for c in range(8):
        b = c // 2
        for ui in range(2):
            r = 2 * (c % 2) + ui
            out[b, r::4, :] = results[c]["out"][ui]
    return out


def run(inputs, trace=False, **spmd_kwargs):
    """Full pipeline; returns (output, BassKernelResults)."""
    nc = _get_nc()
    in_maps = make_in_maps(**inputs)
    res = run_bass_kernel_spmd(
        nc, in_maps, core_ids=list(range(8)), trace=trace, **spmd_kwargs
    )
    return assemble_out(res.results), res


def kernel(**inputs):
    out, _ = run(inputs)
    return out
